# revision 8
# baseline (speedup 1.0000x reference)
"""GRU encoder kernel for Trainium2 (8 NeuronCores, data-parallel over batch).

Problem: B=64, T=512, E=512, H=512 Keras-v2 GRU (gates z,r,h; reset_after).
  x_proj = src @ Wx + b                       [B, T, 3H]
  per step: rec = h @ Wh
            z = sig(xp_z + rec_z); r = sig(xp_r + rec_r)
            hh = tanh(xp_c + r * rec_c)
            h = z*h + (1-z)*hh
Returns (last, hs, last) with hs: [T, B, H].

Per-core design (8 sequences = 2 phase-shifted streams x 4 seqs):
- Phase 1: x_proj via f32r matmuls (PE-transposed src tiles), written to DRAM
  in step-ready layouts (xp_zr bf16 quarter-major for the augmented matmul;
  xp_c f32 strip-major for the DVE add).
- Phase 2: T sequential steps. rec matmul in bf16 with 4-way column-tiled
  concurrent streams (col-group j computes gate-quarters {z_j|r_j|c_j},
  N=384) + one augmented matmul per group adding xp_zr & b via identity
  weights. Elementwise tail in sparse-strip layout [lanes 32j+0..3, 128].
  h -> hT via 4 row-tiled PE transposes. Two phase-shifted streams hide the
  per-step latency chain behind each other's matmuls.
"""
import sys
import numpy as np

sys.path.insert(0, "/opt/trn_rl_repo")

import ml_dtypes  # noqa: E402
import concourse.bass as bass  # noqa: E402
import concourse.tile as tile  # noqa: E402
from concourse import mybir, bacc  # noqa: E402

F32 = mybir.dt.float32
F32R = mybir.dt.float32r
BF16 = mybir.dt.bfloat16

B, T_FULL, E, H = 64, 512, 512, 512
NCORES = 8
BC = B // NCORES          # 8 sequences per core
NS = 2                    # streams per core
NB = BC // NS             # 4 seqs per stream
NJ = 4                    # col groups / H quarters
HQ = H // NJ              # 128
WIN = 8                   # steps buffered per DMA window
ALU = mybir.AluOpType
ACTF = mybir.ActivationFunctionType


def build(T=T_FULL):
    nc = bacc.Bacc("TRN2", target_bir_lowering=False, debug=False)

    src_d = nc.dram_tensor("src", [BC, T, E], F32, kind="ExternalInput")
    wx_d = nc.dram_tensor("wx", [4, 128, 3 * H], F32, kind="ExternalInput")
    whb_d = nc.dram_tensor("whb", [4, 128, 3 * H], BF16, kind="ExternalInput")
    bzr_d = nc.dram_tensor("bzr", [1, 1024], BF16, kind="ExternalInput")
    bc_d = nc.dram_tensor("bc", [1, H], F32, kind="ExternalInput")
    auglhs_d = nc.dram_tensor("auglhs", [NB + 1, NB], BF16, kind="ExternalInput")
    id4_d = nc.dram_tensor("id4", [NB, NB], F32, kind="ExternalInput")
    idT_d = nc.dram_tensor("idT", [128, 128], F32, kind="ExternalInput")
    hs_d = nc.dram_tensor("hs_out", [T, BC, H], F32, kind="ExternalOutput")
    xpzr_d = nc.dram_tensor("xp_zr", [T, NS, NB, 1024], BF16, kind="Internal")
    xpc_d = nc.dram_tensor("xp_c", [T, NS, NJ, NB, HQ], F32, kind="Internal")

    with tile.TileContext(nc) as tc:
        _emit(nc, tc, T, src_d, wx_d, whb_d, bzr_d, bc_d, auglhs_d, id4_d,
              idT_d, hs_d, xpzr_d, xpc_d)
    nc.compile()
    return nc


def _emit(nc, tc, T, src_d, wx_d, whb_d, bzr_d, bc_d, auglhs_d, id4_d,
          idT_d, hs_d, xpzr_d, xpc_d):
    from contextlib import ExitStack
    ctx = ExitStack()
    TT = min(128, T)          # tokens per phase-1 tile
    NT = (T + TT - 1) // TT

    const = ctx.enter_context(tc.tile_pool(name="const", bufs=1))
    wpool = ctx.enter_context(tc.tile_pool(name="weights", bufs=1))

    # ---- constants ----
    auglhs = const.tile([NB + 1, NB], BF16, tag="auglhs")
    nc.gpsimd.dma_start(auglhs[:, :], auglhs_d.ap())
    ident = const.tile([128, NB], F32, tag="ident")
    for j in range(NJ):
        nc.gpsimd.dma_start(ident[32 * j:32 * j + NB, :], id4_d.ap())
    idT = const.tile([128, TT], F32, tag="idT")
    nc.gpsimd.dma_start(idT[:, :], idT_d.ap()[0:128, 0:TT])
    bc_t = const.tile([128, H], F32, tag="bc_t")
    nc.gpsimd.dma_start(bc_t[:, :], bc_d.ap().broadcast_to([128, H]))

    # ---- persistent weights ----
    wh = []
    for k in range(4):
        t = wpool.tile([128, 3 * H], BF16, tag=f"wh{k}", name=f"wh{k}")
        nc.gpsimd.dma_start(t[:, :], whb_d.ap()[k])
        wh.append(t)
    wxr = []
    for k in range(4):
        t = wpool.tile([128, 3 * H], F32R, tag=f"wxr{k}", name=f"wxr{k}")
        wxr.append(t)

    # ================= PHASE 1: x_proj =================
    with (
        tc.tile_pool(name="p1sb", bufs=3) as p1sb,
        tc.tile_pool(name="p1ps", bufs=2, space="PSUM") as p1ps,
    ):
        for k in range(4):
            t0 = p1sb.tile([128, 3 * H], F32, tag="wx0", name=f"wx0_{k}")
            nc.gpsimd.dma_start(t0[:, :], wx_d.ap()[k])
            nc.vector.tensor_copy(wxr[k][:, :], t0[:, :])

        for brow in range(BC):
            s, r = brow // NB, brow % NB
            for tt in range(NT):
                tok0 = tt * TT
                stile = p1sb.tile([128, E], F32, tag="stile", name=f"s_{brow}_{tt}")
                nc.gpsimd.dma_start(stile[0:TT, :], src_d.ap()[brow, tok0:tok0 + TT, :])
                pT = p1ps.tile([128, 4 * TT], F32, tag="pT", name=f"pT_{brow}_{tt}")
                for k in range(4):
                    nc.tensor.transpose(pT[:, TT * k:TT * (k + 1)],
                                        stile[0:TT, 128 * k:128 * (k + 1)],
                                        idT[0:TT, 0:TT])
                sT = p1sb.tile([128, 4 * TT], F32R, tag="sT", name=f"sT_{brow}_{tt}")
                nc.vector.tensor_copy(sT[:, :], pT[:, :])
                banks = []
                for n in range(3):
                    p = p1ps.tile([128, 512], F32, tag=f"xpb{n}", name=f"xpb{n}_{brow}_{tt}")
                    for k in range(4):
                        nc.tensor.matmul(p[0:TT, :], sT[:, TT * k:TT * (k + 1)],
                                         wxr[k][:, 512 * n:512 * (n + 1)],
                                         start=(k == 0), stop=(k == 3))
                    banks.append(p)
                zrst = p1sb.tile([128, 1024], BF16, tag="zrst", name=f"zrst_{brow}_{tt}")
                zr3 = zrst[0:TT, :].rearrange("p (j g u) -> p g j u", j=NJ, g=2)
                for g in range(2):
                    nc.vector.tensor_copy(
                        zr3[:, g],
                        banks[g][0:TT, :].rearrange("p (j u) -> p j u", j=NJ))
                cst = p1sb.tile([128, 512], F32, tag="cst", name=f"cst_{brow}_{tt}")
                nc.vector.tensor_tensor(cst[0:TT, :], banks[2][0:TT, :],
                                        bc_t[0:TT, :], ALU.add)
                nc.gpsimd.dma_start(xpzr_d.ap()[tok0:tok0 + TT, s, r, :], zrst[0:TT, :])
                nc.gpsimd.dma_start(
                    xpc_d.ap()[tok0:tok0 + TT, s, :, r, :],
                    cst[0:TT, :].rearrange("p (j u) -> p j u", j=NJ))

    # ================= PHASE 2: recurrence =================
    p2 = ctx.enter_context(tc.tile_pool(name="p2", bufs=1))
    gp = ctx.enter_context(tc.tile_pool(name="gates", bufs=3))
    pspool = ctx.enter_context(tc.tile_pool(name="ps2", bufs=1, space="PSUM"))
    Rb = [[pspool.tile([128, 512], F32, tag=f"Rb{s}_{pp}", name=f"Rb{s}_{pp}")
           for pp in range(2)] for s in range(NS)]
    # one 4-bank transpose staging tile shared by both streams: transpose j
    # drains into its own bank (concurrent row-group transposes into one
    # bank lock up the device).
    Tq = pspool.tile([128, 2048], F32, tag="Tq")
    nc.vector.memset(Tq[:, :], 0.0)
    for s in range(NS):
        for pp in range(2):
            nc.vector.memset(Rb[s][pp][:, :], 0.0)

    NWIN = (T + WIN - 1) // WIN
    hT = [[p2.tile([128, 4 * NB], BF16, tag=f"hT{s}_{pp}", name=f"hT{s}_{pp}")
           for pp in range(2)] for s in range(NS)]
    hsacc = [[p2.tile([128, WIN * HQ], F32, tag=f"hsacc{s}_{pp}", name=f"hsacc{s}_{pp}")
              for pp in range(2)] for s in range(NS)]
    xpzr = [[p2.tile([NB + 1, WIN * 1024], BF16, tag=f"xpzr{s}_{pp}", name=f"xpzr{s}_{pp}")
             for pp in range(2)] for s in range(NS)]
    xpc = [[p2.tile([128, WIN * HQ], F32, tag=f"xpc{s}_{pp}", name=f"xpc{s}_{pp}")
            for pp in range(2)] for s in range(NS)]
    hzero = p2.tile([128, HQ], F32, tag="hzero")
    nc.vector.memset(hzero[:, :], 0.0)
    for s in range(NS):
        for pp in range(2):
            nc.vector.memset(hT[s][pp][:, :], 0.0)
            nc.vector.memset(hsacc[s][pp][:, :], 0.0)
            nc.vector.memset(xpc[s][pp][:, :], 0.0)
            nc.gpsimd.dma_start(
                xpzr[s][pp][NB:NB + 1, :].rearrange("p (w f) -> p w f", w=WIN),
                bzr_d.ap().rearrange("p f -> p () f").broadcast_to([1, WIN, 1024]))


    def load_window(s, w, pp):
        t0 = w * WIN
        n = min(WIN, T - t0)
        nc.gpsimd.dma_start(
            xpzr[s][pp][0:NB, 0:n * 1024].rearrange("p (w f) -> p w f", w=n),
            xpzr_d.ap()[t0:t0 + n, s].rearrange("t r f -> r t f"))
        for j in range(NJ):
            nc.gpsimd.dma_start(
                xpc[s][pp][32 * j:32 * j + NB, 0:n * HQ].rearrange("p (w u) -> p w u", w=n),
                xpc_d.ap()[t0:t0 + n, s, j].rearrange("t r u -> r t u"))

    def store_window(s, w, pp):
        t0 = w * WIN
        n = min(WIN, T - t0)
        for j in range(NJ):
            nc.gpsimd.dma_start(
                hs_d.ap()[t0:t0 + n, NB * s:NB * (s + 1), HQ * j:HQ * (j + 1)].rearrange("t r u -> r t u"),
                hsacc[s][pp][32 * j:32 * j + NB, 0:n * HQ].rearrange("p (w u) -> p w u", w=n))

    for s in range(NS):
        load_window(s, 0, 0)
        if NWIN > 1:
            load_window(s, 1, 1)

    def mm_step(s, t):
        pp = (t // WIN) % 2
        slot = t % WIN
        R = Rb[s][t % 2]
        lhs = hT[s][t % 2]
        for j in range(NJ):
            for k in range(4):
                nc.tensor.matmul(
                    R[32 * j:32 * j + NB, 0:384], lhs[:, NB * k:NB * (k + 1)],
                    wh[k][:, :].rearrange("p (g q) -> p g q", g=3)[:, :, HQ * j:HQ * (j + 1)],
                    start=(k == 0), stop=False, tile_position=(0, 32 * j),
                    skip_group_check=True)
            nc.tensor.matmul(R[32 * j:32 * j + NB, 0:256], auglhs[0:NB + 1, :],
                             xpzr[s][pp][0:NB + 1, slot * 1024 + 256 * j: slot * 1024 + 256 * (j + 1)],
                             start=False, stop=True, tile_position=(0, 32 * j),
                             skip_group_check=True)
        return R

    def transpose_h(s, t):
        pp = (t // WIN) % 2
        slot = t % WIN
        for j in range(NJ):
            nc.tensor.transpose(Tq[:, 512 * j:512 * j + NB],
                                hsacc[s][pp][32 * j:32 * j + NB, slot * HQ:(slot + 1) * HQ],
                                ident[32 * j:32 * j + NB, :],
                                tile_position=(32 * j, 0))
        nc.vector.tensor_copy(
            hT[s][(t + 1) % 2][:, :].rearrange("p (j c) -> p j c", j=NJ),
            Tq[:, :].rearrange("p (j x) -> p j x", j=NJ)[:, :, 0:NB])

    def tail(s, t, R):
        pp = (t // WIN) % 2
        slot = t % WIN
        zr = gp.tile([128, 256], F32, tag=f"zr{s}", name=f"zr{s}_{t}")
        z_t, r_t = zr[:, 0:128], zr[:, 128:256]
        nc.scalar.activation(r_t, R[:, 128:256], ACTF.Sigmoid)
        nc.scalar.activation(z_t, R[:, 0:128], ACTF.Sigmoid)
        sc = gp.tile([128, HQ], F32, tag=f"sc{s}", name=f"sc{s}_{t}")
        nc.vector.tensor_tensor(sc[:, :], r_t, R[:, 256:384], ALU.mult)
        nc.vector.tensor_tensor(sc[:, :], sc[:, :],
                                xpc[s][pp][:, slot * HQ:(slot + 1) * HQ], ALU.add)
        hh = gp.tile([128, HQ], F32, tag=f"hh{s}", name=f"hh{s}_{t}")
        nc.scalar.activation(hh[:, :], sc[:, :], ACTF.Tanh)
        if t == 0:
            hprev = hzero[:, :]
        else:
            hprev = hsacc[s][((t - 1) // WIN) % 2][:, ((t - 1) % WIN) * HQ:((t - 1) % WIN) * HQ + HQ]
        u = gp.tile([128, HQ], F32, tag=f"u{s}", name=f"u{s}_{t}")
        nc.vector.tensor_tensor(u[:, :], z_t, hprev, ALU.mult)
        v = gp.tile([128, HQ], F32, tag=f"v{s}", name=f"v{s}_{t}")
        nc.vector.scalar_tensor_tensor(v[:, :], z_t, 1.0, hh[:, :],
                                       op0=ALU.subtract, op1=ALU.mult)
        nc.vector.tensor_tensor(hsacc[s][pp][:, slot * HQ:(slot + 1) * HQ],
                                u[:, :], v[:, :], ALU.subtract)

    for t in range(T):
        for s in range(NS):
            R = mm_step(s, t)
            tail(s, t, R)
            if t + 1 < T:
                transpose_h(s, t)
        w = t // WIN
        if t % WIN == WIN - 1:
            for s in range(NS):
                store_window(s, w, w % 2)
                if w + 2 < NWIN:
                    load_window(s, w + 2, w % 2)
    ctx.close()


def _prep_inputs(src_shard, Wx, Wh, b):
    bf = ml_dtypes.bfloat16
    wx = np.ascontiguousarray(Wx.reshape(4, 128, 3 * H)).astype(np.float32)
    whb = np.ascontiguousarray(Wh.reshape(4, 128, 3 * H)).astype(bf)
    bzr = np.concatenate(
        [np.concatenate([b[HQ * j:HQ * (j + 1)], b[H + HQ * j:H + HQ * (j + 1)]])
         for j in range(NJ)]).reshape(1, 1024).astype(bf)
    bc = b[2 * H:].reshape(1, H).astype(np.float32)
    auglhs = np.concatenate([np.eye(NB), np.ones((1, NB))]).astype(bf)
    id4 = np.eye(NB, dtype=np.float32)
    idT = np.eye(128, dtype=np.float32)
    return {
        "src": np.ascontiguousarray(src_shard).astype(np.float32),
        "wx": wx, "whb": whb, "bzr": bzr, "bc": bc,
        "auglhs": auglhs, "id4": id4, "idT": idT,
    }


_NC_CACHE = {}


def _get_nc(T):
    if T not in _NC_CACHE:
        _NC_CACHE[T] = build(T)
    return _NC_CACHE[T]


def kernel(src_seq, Wx, Wh, b):
    from concourse.bass_utils import run_bass_kernel_spmd

    src_seq = np.asarray(src_seq, dtype=np.float32)
    Wx = np.asarray(Wx, dtype=np.float32)
    Wh = np.asarray(Wh, dtype=np.float32)
    b = np.asarray(b, dtype=np.float32)
    T = src_seq.shape[1]

    nc = _get_nc(T)
    in_maps = [_prep_inputs(src_seq[BC * i:BC * (i + 1)], Wx, Wh, b)
               for i in range(NCORES)]
    res = run_bass_kernel_spmd(nc, in_maps, core_ids=list(range(NCORES)))
    hs = np.concatenate([r["hs_out"] for r in res.results], axis=1)
    last = hs[-1].copy()
    return last, hs, last
